# revision 1
# baseline (speedup 1.0000x reference)
"""Causal self-attention on 8 Trainium2 NeuronCores.

Sharding: B*H = 2*12 = 24 (batch, head) pairs -> 3 heads per core.
Core i handles batch i//4, heads 3*(i%4) .. 3*(i%4)+2.
Each core computes q/k/v projections for its 3 heads (tensor-parallel slice
of wq/wk/wv), causal attention, and a partial out-projection against its
192 columns of wo. Host sums the 4 partials per batch (the "all-reduce").

Per-core kernel (all fp32 data; matmuls run as float32r = full-rate fp32):
  - x [T, 768] loaded in natural layout, PE-transposed to xT tiles.
  - qT/kT computed in [64, T] layout; v computed via vT then PE-transposed
    to natural [T, 64] with a ones column appended (for softmax denominators).
  - S_T[kblock, qblock] = K_blk @ Q_blk.T  (contraction d=64)
  - P_T = exp(S_T / 8)   on ACT, grouped over kblocks for large free dims
  - causal masking: multiply diagonal-crossing blocks by a 0/1 triangle mask
  - attnU_T[65, TQ] += Vaug_blk.T @ P_T   (row 64 = softmax denominator)
  - divide via reciprocal + partition-broadcast, then
    y[T,768] partial = attnT.T @ woT_slice.
No max-subtraction in softmax: logits here have |.| <~ 2, exp is safe, and
softmax(x) == exp(x)/sum(exp(x)) exactly.

Partition-base alignment (matmul requires lhsT/rhs at the same base
partition; ACT/DVE copies are lane-aligned): per-head pairs live at the
same partition offset:
  q01 [128,T] = qT_h0 (rows 0:64) | qT_h1 (rows 64:128)
  k01 [128,T] = kT_h0 | kT_h1
  qv0 [128,T] = qT_h2 | vT_h0
  kv1 [128,T] = kT_h2 | vT_h1
  v2t [64,T]  = vT_h2
S for h0: (k01[0:64], q01[0:64]); h1: (k01[64:128], q01[64:128]);
h2: (kv1[0:64], qv0[0:64]).
"""

import numpy as np

import concourse.bass as bass
import concourse.mybir as mybir
from concourse import bacc
from concourse import tile
from concourse.bass_utils import run_bass_kernel_spmd
from concourse.masks import make_identity

F32 = mybir.dt.float32
F32R = mybir.dt.float32r

EMBED = 768
NHEAD = 12
DH = 64
B = 2
T = 4096
HPC = 3          # heads per core
CH = HPC * DH    # 192 channels per core
NCORES = 8


def build_program(t=T, debug_taps=False):
    """Build the single-core SPMD Bass program."""
    nqb = t // 256   # q blocks of 256
    ntb = t // 512   # projection T-blocks of 512

    nc = bacc.Bacc("TRN2", target_bir_lowering=False, debug=False,
                   num_devices=NCORES)

    x_d = nc.dram_tensor("x", [t, EMBED], F32, kind="ExternalInput")
    # columns: q0,q1 | k0,k1 | q2,v0 | k2,v1 | v2   (64 each)
    wqkv_d = nc.dram_tensor("wqkvT", [EMBED, 576], F32, kind="ExternalInput")
    bqkv_d = nc.dram_tensor("bqkv", [576, 1], F32, kind="ExternalInput")
    wo_d = nc.dram_tensor("woT", [CH, EMBED], F32, kind="ExternalInput")
    y_d = nc.dram_tensor("y", [t, EMBED], F32, kind="ExternalOutput")
    if debug_taps:
        dbg_q01 = nc.dram_tensor("dbg_q01", [128, t], F32,
                                 kind="ExternalOutput")
        dbg_k01 = nc.dram_tensor("dbg_k01", [128, t], F32,
                                 kind="ExternalOutput")
        dbg_vs0 = nc.dram_tensor("dbg_vs0", [128, (t // 128) * 65], F32,
                                 kind="ExternalOutput")
        dbg_pt = nc.dram_tensor("dbg_pt", [128, 512], F32,
                                kind="ExternalOutput")
        dbg_rec = nc.dram_tensor("dbg_rec", [1, 256], F32,
                                 kind="ExternalOutput")
        dbg_attn = nc.dram_tensor("dbg_attn", [192, 256], F32,
                                  kind="ExternalOutput")

    Act = mybir.ActivationFunctionType

    with tile.TileContext(nc) as tc:
        with (
            tc.tile_pool(name="const", bufs=1) as cpool,
            tc.tile_pool(name="persist", bufs=1) as perm,
        ):
            ident = cpool.tile([128, 128], F32, tag="ident")
            make_identity(nc, ident)
            # bigmask[si, u] = 1.0 if si <= u - 128 else 0.0
            # diag kblock (d=0)  -> slice [:, 128:384];  d=-128 -> [:, 0:256]
            # all-ones row at partition 64 (for denominator broadcast mm)
            ones65 = cpool.tile([65, 64], F32R, tag="ones65")
            nc.gpsimd.memset(ones65.bitcast(F32), 1.0)
            bigmask = cpool.tile([128, 384], F32, tag="bigmask")
            nc.gpsimd.memset(bigmask, 1.0)
            nc.gpsimd.affine_select(
                out=bigmask, in_=bigmask,
                compare_op=mybir.AluOpType.is_ge, fill=0.0,
                base=-128, pattern=[[1, 384]], channel_multiplier=-1,
            )

            # weights
            wqkv_sb = []
            for kt in range(6):
                w_raw = cpool.tile([128, 576], F32, name=f"wqkvraw{kt}",
                                   tag=f"wqkvraw{kt}")
                nc.sync.dma_start(w_raw, wqkv_d[kt * 128:(kt + 1) * 128, :])
                w_t = cpool.tile([128, 576], F32R, name=f"wqkv{kt}",
                                 tag=f"wqkv{kt}")
                nc.vector.tensor_copy(w_t, w_raw)
                wqkv_sb.append(w_t)
            bias_sb = []
            for mc in range(5):
                mw = 128 if mc < 4 else 64
                b_t = cpool.tile([128, 1], F32, name=f"bias{mc}",
                                 tag=f"bias{mc}")
                nc.sync.dma_start(b_t[:mw, :],
                                  bqkv_d[mc * 128:mc * 128 + mw, :])
                bias_sb.append(b_t)
            wo_sb = []
            for h in range(3):
                wo_raw = cpool.tile([64, EMBED], F32, name=f"woraw{h}",
                                    tag=f"woraw{h}")
                nc.sync.dma_start(wo_raw, wo_d[h * 64:(h + 1) * 64, :])
                wo_h = cpool.tile([64, EMBED], F32R, name=f"wo{h}",
                                  tag=f"wo{h}")
                nc.vector.tensor_copy(wo_h, wo_raw)
                wo_sb.append(wo_h)

            # persistent activations
            q01 = perm.tile([128, t], F32R, tag="q01")
            k01 = perm.tile([128, t], F32R, tag="k01")
            qv0 = perm.tile([128, t], F32R, tag="qv0")
            kv1 = perm.tile([128, t], F32R, tag="kv1")
            v2t = perm.tile([64, t], F32R, tag="v2t")
            # v natural, 65-wide per 128-row chunk (col 64 = ones)
            vs = [perm.tile([128, (t // 128) * 65], F32R, name=f"vs{h}",
                            tag=f"vs{h}")
                  for h in range(3)]
            for h in range(3):
                nc.gpsimd.memset(vs[h].bitcast(F32), 1.0)

            proj_dest = [q01, k01, qv0, kv1, v2t]
            # (qT slice, kT slice) per head for the S matmul
            def q_ap(h):
                return (q01[0:64], q01[64:128], qv0[0:64])[h]

            def k_ap(h):
                return (k01[0:64], k01[64:128], kv1[0:64])[h]

            # ---------------- phase A: projections ----------------
            with (
                tc.tile_pool(name="xpool", bufs=4) as xpool,
                tc.tile_pool(name="xtpool", bufs=2) as xtpool,
                tc.tile_pool(name="tpsum", bufs=6, space="PSUM") as tpsum,
                tc.tile_pool(name="projpsum", bufs=2, space="PSUM") as projpsum,
            ):
                for tb in range(ntb):
                    xts = [xtpool.tile([128, 512], F32R, tag=f"xt{ct}",
                                       name=f"xt{ct}_{tb}")
                           for ct in range(6)]
                    for i in range(4):
                        row0 = tb * 512 + i * 128
                        xn = xpool.tile([128, EMBED], F32, tag="xn",
                                        name=f"xn{tb}_{i}")
                        nc.sync.dma_start(xn, x_d[row0:row0 + 128, :])
                        for ct in range(6):
                            tp = tpsum.tile([128, 128], F32, tag="tp",
                                            name=f"tp{tb}_{i}_{ct}")
                            nc.tensor.transpose(
                                tp, xn[:, ct * 128:(ct + 1) * 128], ident)
                            nc.vector.tensor_copy(
                                xts[ct][:, i * 128:(i + 1) * 128], tp)
                    for mc in range(5):
                        mw = 128 if mc < 4 else 64
                        ps = projpsum.tile([mw, 512], F32, tag="proj",
                                           name=f"proj{tb}_{mc}")
                        for ct in range(6):
                            nc.tensor.matmul(
                                ps,
                                lhsT=wqkv_sb[ct][:, mc * 128:mc * 128 + mw],
                                rhs=xts[ct],
                                start=(ct == 0), stop=(ct == 5))
                        dest = proj_dest[mc][:, tb * 512:(tb + 1) * 512]
                        nc.scalar.activation(dest, ps, Act.Identity,
                                             bias=bias_sb[mc][:mw, :],
                                             scale=1.0)

                # ---------------- phase B: v transpose ----------------
                v_src = [qv0[64:128], kv1[64:128], v2t[0:64]]
                v_idn = [ident[64:128, 64:128], ident[64:128, 64:128],
                         ident[0:64, 0:64]]
                for h in range(3):
                    for ck in range(t // 128):
                        tp2 = tpsum.tile([128, 64], F32, tag="tp",
                                         name=f"vt{h}_{ck}")
                        nc.tensor.transpose(
                            tp2,
                            v_src[h][:, ck * 128:(ck + 1) * 128].bitcast(F32),
                            v_idn[h])
                        nc.vector.tensor_copy(
                            vs[h][:, ck * 65:ck * 65 + 64], tp2)

            if debug_taps:
                nc.sync.dma_start(dbg_q01[:, :], q01.bitcast(F32))
                nc.sync.dma_start(dbg_k01[:, :], k01.bitcast(F32))
                nc.sync.dma_start(dbg_vs0[:, :], vs[0].bitcast(F32))

            # ---------------- phase C/D: attention + out-proj ----------------
            with (
                tc.tile_pool(name="spsum", bufs=2, space="PSUM") as spsum,
                tc.tile_pool(name="accpsum", bufs=1, space="PSUM") as accpsum,
                tc.tile_pool(name="bcpsum", bufs=1, space="PSUM") as bcpsum,
                tc.tile_pool(name="ypsum", bufs=1, space="PSUM") as ypsum,
                tc.tile_pool(name="ppool", bufs=3) as ppool,
                tc.tile_pool(name="apool", bufs=3) as apool,
                tc.tile_pool(name="rpool", bufs=4) as rpool,
                tc.tile_pool(name="ysb", bufs=3) as ysb,
            ):
                for qb in range(nqb):
                    q_sl = slice(qb * 256, (qb + 1) * 256)
                    attn = [apool.tile([64, 256], F32R, tag=f"attn{h}",
                                       name=f"attn{h}_{qb}")
                            for h in range(3)]
                    kbn = 2 * qb + 2
                    for h in range(3):
                        acc = accpsum.tile([65, 256], F32, tag="acc",
                                           name=f"acc{qb}_{h}")
                        ngroups = (kbn + 3) // 4
                        for g in range(ngroups):
                            gk = min(4, kbn - g * 4)
                            sp = spsum.tile([128, gk * 256], F32, tag="s",
                                            name=f"s{qb}_{h}_{g}")
                            pt = ppool.tile([128, gk * 256], F32R, tag="p",
                                            name=f"p{qb}_{h}_{g}")
                            for j in range(gk):
                                kbi = g * 4 + j
                                nc.tensor.matmul(
                                    sp[:, j * 256:(j + 1) * 256],
                                    lhsT=k_ap(h)[:,
                                                 kbi * 128:(kbi + 1) * 128],
                                    rhs=q_ap(h)[:, q_sl],
                                    start=True, stop=True)
                            nc.scalar.activation(pt, sp, Act.Exp,
                                                 bias=0.0, scale=0.125)
                            for j in range(gk):
                                kbi = g * 4 + j
                                if kbi == 2 * qb:
                                    nc.vector.tensor_mul(
                                        pt[:, j * 256:(j + 1) * 256],
                                        pt[:, j * 256:(j + 1) * 256],
                                        bigmask[:, 128:384])
                                elif kbi == 2 * qb + 1:
                                    nc.vector.tensor_mul(
                                        pt[:, j * 256:(j + 1) * 256],
                                        pt[:, j * 256:(j + 1) * 256],
                                        bigmask[:, 0:256])
                            if debug_taps and qb == 0 and h == 0 and g == 0:
                                nc.sync.dma_start(dbg_pt[:, :gk * 256],
                                                  pt.bitcast(F32))
                            for j in range(gk):
                                kbi = g * 4 + j
                                nc.tensor.matmul(
                                    acc,
                                    lhsT=vs[h][:, kbi * 65:kbi * 65 + 65],
                                    rhs=pt[:, j * 256:(j + 1) * 256],
                                    start=(kbi == 0), stop=(kbi == kbn - 1))
                        # epilogue: copy acc out of PSUM immediately (frees
                        # the bank for the next head), then divide by the
                        # denominators.  The denom row sits at partition 64
                        # (DVE lanes are partition-fixed), so broadcast
                        # 1/denom across partitions 0:64 with a K=1 matmul
                        # whose operands both live at base partition 64.
                        acc_sb = rpool.tile([65, 256], F32, tag="accsb",
                                            name=f"accsb{qb}_{h}")
                        nc.vector.tensor_copy(acc_sb, acc)
                        rec = rpool.tile([65, 256], F32R, tag="rec",
                                         name=f"rec{qb}_{h}")
                        with nc.allow_low_precision(
                                reason="fp32r operand rounding"):
                            nc.vector.reciprocal(rec[64:65, :],
                                                 acc_sb[64:65, :])
                        bc = bcpsum.tile([64, 256], F32, tag="bc",
                                         name=f"bc{qb}_{h}")
                        nc.tensor.matmul(bc, lhsT=ones65[64:65, :],
                                         rhs=rec[64:65, :],
                                         start=True, stop=True)
                        nc.vector.tensor_mul(attn[h], acc_sb[0:64, :], bc)
                        if debug_taps and qb == 0:
                            if h == 0:
                                nc.sync.dma_start(dbg_rec[:, :],
                                                  rec[64:65, :])
                            nc.sync.dma_start(
                                dbg_attn[h * 64:(h + 1) * 64, :],
                                attn[h].bitcast(F32))
                    # out-projection for this q block
                    for mt in range(2):
                        yp = ypsum.tile([128, EMBED], F32, tag="y",
                                        name=f"y{qb}_{mt}")
                        t_sl = slice(mt * 128, (mt + 1) * 128)
                        for n0, nw in ((0, 512), (512, 256)):
                            for h in range(3):
                                nc.tensor.matmul(
                                    yp[:, n0:n0 + nw],
                                    lhsT=attn[h][:, t_sl],
                                    rhs=wo_sb[h][:, n0:n0 + nw],
                                    start=(h == 0), stop=(h == 2))
                        ys = ysb.tile([128, EMBED], F32, tag="ys",
                                      name=f"ys{qb}_{mt}")
                        nc.vector.tensor_copy(ys, yp)
                        row0 = qb * 256 + mt * 128
                        nc.sync.dma_start(y_d[row0:row0 + 128, :], ys)
    nc.compile()
    return nc


_PROG_CACHE = {}


def _get_program(t=T):
    if t not in _PROG_CACHE:
        _PROG_CACHE[t] = build_program(t)
    return _PROG_CACHE[t]


def make_in_maps(x, wq, bq, wk, bk, wv, bv, wo):
    in_maps = []
    for core in range(NCORES):
        b = core // 4
        hs = (core % 4) * HPC
        sl = [slice((hs + h) * DH, (hs + h + 1) * DH) for h in range(HPC)]
        # columns: q0,q1 | k0,k1 | q2,v0 | k2,v1 | v2
        cols = [wq[sl[0]].T, wq[sl[1]].T, wk[sl[0]].T, wk[sl[1]].T,
                wq[sl[2]].T, wv[sl[0]].T, wk[sl[2]].T, wv[sl[1]].T,
                wv[sl[2]].T]
        biases = [bq[sl[0]], bq[sl[1]], bk[sl[0]], bk[sl[1]],
                  bq[sl[2]], bv[sl[0]], bk[sl[2]], bv[sl[1]], bv[sl[2]]]
        wqkvT = np.ascontiguousarray(np.concatenate(cols, axis=1),
                                     dtype=np.float32)
        bqkv = np.ascontiguousarray(
            np.concatenate(biases)[:, None], dtype=np.float32)
        ch = slice(hs * DH, (hs + HPC) * DH)
        woT = np.ascontiguousarray(wo[:, ch].T, dtype=np.float32)
        in_maps.append({
            "x": np.ascontiguousarray(x[b], dtype=np.float32),
            "wqkvT": wqkvT,
            "bqkv": bqkv,
            "woT": woT,
        })
    return in_maps


def run(inputs, t=T, trace=False, **kw):
    """Run on hardware; returns (y, BassKernelResults)."""
    arrs = {k: np.asarray(v, dtype=np.float32) for k, v in inputs.items()}
    nc = _get_program(t)
    in_maps = make_in_maps(**arrs)
    res = run_bass_kernel_spmd(nc, in_maps, list(range(NCORES)),
                               trace=trace, **kw)
    outs = [np.asarray(m["y"], dtype=np.float32) for m in res.results]
    y = np.empty((B, t, EMBED), dtype=np.float32)
    for b in range(B):
        y[b] = outs[4 * b] + outs[4 * b + 1] + outs[4 * b + 2] + outs[4 * b + 3]
    return y, res


def kernel(**inputs):
    y, _ = run(inputs)
    return y



# revision 7
# speedup vs baseline: 4777.5649x; 4777.5649x over previous
"""Causal self-attention on 8 Trainium2 NeuronCores.

Sharding: B*H = 2*12 = 24 (batch, head) pairs -> 3 heads per core.
Core i handles batch i//4, heads 3*(i%4) .. 3*(i%4)+2.
Each core computes q/k/v projections for its 3 heads (tensor-parallel slice
of wq/wk/wv), causal attention, and a partial out-projection against its
192 columns of wo. Host sums the 4 partials per batch (the "all-reduce").

v2 (speed over v1 baseline):
  - Attention loop is software-pipelined: the S matmuls for group g+1 are
    emitted BEFORE the PV matmuls of group g, so the PE never sits behind
    the ACT exp on its in-order queue; epilogues (reciprocal/broadcast/div)
    and the out-projection are deferred one/two stages for the same reason.
    Keeping the PE continuously busy also holds the HAM clock gate at
    2.4 GHz instead of 1.2.
  - Causal masking via gpsimd affine_select on the two diagonal blocks of
    every q-block (Pool engine, otherwise idle) instead of DVE mask
    multiplies; the affine params are the same for every unit
    (keep iff u - si >= 0 / u - si - 128 >= 0).
  - x-transpose PSUM->SBUF copies split between ACT and DVE; fp32 weights
    are bitcast to fp32r (no copy).
  - v transposes are interleaved into phase A per 512-row block.
  - Out-projection column-split into two 384-wide PSUM tiles for natural
    double buffering.

Per-core kernel (all fp32 data; matmuls run as float32r = full-rate fp32):
  - x [T, 768] loaded in natural layout, PE-transposed to xT tiles.
  - qT/kT computed in [64, T] layout; v computed via vT then PE-transposed
    to natural [T, 64] with a ones column appended (softmax denominators).
  - S_T[kblock, qblock] = K_blk @ Q_blk.T  (contraction d=64)
  - P_T = exp(S_T / 8)  on ACT, grouped over <=4 kblocks
  - attnU_T[65, TQ] += Vaug_blk.T @ P_T  (row 64 = softmax denominator)
  - divide via reciprocal + partition-broadcast matmul, then
    y[T,768] partial = attnT.T @ woT_slice.
No max-subtraction in softmax: logits here have |.| <~ 2, exp is safe.

Partition-base alignment: per-head pairs live at the same partition offset:
  q01 [128,T] = qT_h0 (rows 0:64) | qT_h1 (rows 64:128)
  k01 [128,T] = kT_h0 | kT_h1
  qv0 [128,T] = qT_h2 | vT_h0
  kv1 [128,T] = kT_h2 | vT_h1
  v2t [64,T]  = vT_h2
S for h0: (k01[0:64], q01[0:64]); h1: (k01[64:128], q01[64:128]);
h2: (kv1[0:64], qv0[0:64]).
"""

import numpy as np

import concourse.bass as bass
import concourse.mybir as mybir
from concourse import bacc
from concourse import tile
from concourse.bass_utils import run_bass_kernel_spmd
from concourse.masks import make_identity

F32 = mybir.dt.float32
F32R = mybir.dt.float32r

EMBED = 768
NHEAD = 12
DH = 64
B = 2
T = 4096
HPC = 3          # heads per core
CH = HPC * DH    # 192 channels per core
NCORES = 8


def build_program(t=T):
    """Build the single-core SPMD Bass program."""
    nqb = t // 256   # q blocks of 256
    ntb = t // 512   # projection T-blocks of 512

    nc = bacc.Bacc("TRN2", target_bir_lowering=False, debug=False,
                   num_devices=NCORES)

    x_d = nc.dram_tensor("x", [t, EMBED], F32, kind="ExternalInput")
    # columns: q0,q1 | k0,k1 | q2,v0 | k2,v1 | v2   (64 each)
    wqkv_d = nc.dram_tensor("wqkvT", [EMBED, 576], F32, kind="ExternalInput")
    bqkv_d = nc.dram_tensor("bqkv", [576, 1], F32, kind="ExternalInput")
    wo_d = nc.dram_tensor("woT", [CH, EMBED], F32, kind="ExternalInput")
    y_d = nc.dram_tensor("y", [t, EMBED], F32, kind="ExternalOutput")

    Act = mybir.ActivationFunctionType

    with tile.TileContext(nc) as tc:
        with (
            tc.tile_pool(name="const", bufs=1) as cpool,
            tc.tile_pool(name="persist", bufs=1) as perm,
        ):
            ident = cpool.tile([128, 128], F32, tag="ident")
            make_identity(nc, ident)
            # all-ones rows (for the denominator partition-broadcast mm)
            ones_t = cpool.tile([128, 64], F32R, tag="ones")
            nc.gpsimd.memset(ones_t.bitcast(F32), 1.0)

            # weights (fp32 -> fp32r conversion copies; fp32r is a rounded
            # format, a bitcast does not satisfy the BIR verifier)
            wqkv_sb = []
            for kt in range(6):
                w_raw = cpool.tile([128, 576], F32, name=f"wqkvraw{kt}",
                                   tag=f"wqkvraw{kt}")
                nc.sync.dma_start(w_raw, wqkv_d[kt * 128:(kt + 1) * 128, :])
                w_t = cpool.tile([128, 576], F32R, name=f"wqkv{kt}",
                                 tag=f"wqkv{kt}")
                nc.vector.tensor_copy(w_t, w_raw)
                wqkv_sb.append(w_t)
            bias_sb = []
            for mc in range(5):
                mw = 128 if mc < 4 else 64
                b_t = cpool.tile([128, 1], F32, name=f"bias{mc}",
                                 tag=f"bias{mc}")
                nc.sync.dma_start(b_t[:mw, :],
                                  bqkv_d[mc * 128:mc * 128 + mw, :])
                bias_sb.append(b_t)
            wo_sb = []
            for h in range(3):
                wo_raw = cpool.tile([64, EMBED], F32, name=f"woraw{h}",
                                    tag=f"woraw{h}")
                nc.sync.dma_start(wo_raw, wo_d[h * 64:(h + 1) * 64, :])
                wo_h = cpool.tile([64, EMBED], F32R, name=f"wo{h}",
                                  tag=f"wo{h}")
                nc.vector.tensor_copy(wo_h, wo_raw)
                wo_sb.append(wo_h)

            # persistent activations
            q01 = perm.tile([128, t], F32R, tag="q01")
            k01 = perm.tile([128, t], F32R, tag="k01")
            qv0 = perm.tile([128, t], F32R, tag="qv0")
            kv1 = perm.tile([128, t], F32R, tag="kv1")
            v2t = perm.tile([64, t], F32R, tag="v2t")
            # v natural, 65-wide per 128-row chunk (col 64 = ones)
            vs = [perm.tile([128, (t // 128) * 65], F32R, name=f"vs{h}",
                            tag=f"vs{h}")
                  for h in range(3)]
            for h in range(3):
                nc.gpsimd.memset(vs[h].bitcast(F32), 1.0)

            proj_dest = [q01, k01, qv0, kv1, v2t]

            def q_ap(h):
                return (q01[0:64], q01[64:128], qv0[0:64])[h]

            def k_ap(h):
                return (k01[0:64], k01[64:128], kv1[0:64])[h]

            # ---------------- phase A: projections (+ v transpose) --------
            v_src = [qv0[64:128], kv1[64:128], v2t[0:64]]
            v_idn = [ident[64:128, 64:128], ident[64:128, 64:128],
                     ident[0:64, 0:64]]
            with (
                tc.tile_pool(name="xpool", bufs=4) as xpool,
                tc.tile_pool(name="xtpool", bufs=2) as xtpool,
                tc.tile_pool(name="tpsum", bufs=6, space="PSUM") as tpsum,
                tc.tile_pool(name="projpsum", bufs=2, space="PSUM") as projpsum,
            ):
                for tb in range(ntb):
                    xts = [xtpool.tile([128, 512], F32R, tag=f"xt{ct}",
                                       name=f"xt{ct}_{tb}")
                           for ct in range(6)]
                    for i in range(4):
                        row0 = tb * 512 + i * 128
                        xn = xpool.tile([128, EMBED], F32, tag="xn",
                                        name=f"xn{tb}_{i}")
                        nc.sync.dma_start(xn, x_d[row0:row0 + 128, :])
                        for ct in range(6):
                            tp = tpsum.tile([128, 128], F32, tag="tp",
                                            name=f"tp{tb}_{i}_{ct}")
                            nc.tensor.transpose(
                                tp, xn[:, ct * 128:(ct + 1) * 128], ident)
                            dst = xts[ct][:, i * 128:(i + 1) * 128]
                            if ct < 4:
                                nc.scalar.copy(dst, tp)
                            else:
                                nc.vector.tensor_copy(dst, tp)
                    for mc in range(5):
                        mw = 128 if mc < 4 else 64
                        ps = projpsum.tile([mw, 512], F32, tag="proj",
                                           name=f"proj{tb}_{mc}")
                        for ct in range(6):
                            nc.tensor.matmul(
                                ps,
                                lhsT=wqkv_sb[ct][:, mc * 128:mc * 128 + mw],
                                rhs=xts[ct],
                                start=(ct == 0), stop=(ct == 5))
                        dest = proj_dest[mc][:, tb * 512:(tb + 1) * 512]
                        nc.scalar.activation(dest, ps, Act.Identity,
                                             bias=bias_sb[mc][:mw, :],
                                             scale=1.0)
                    # v transpose for this T-block (4 column chunks of 128)
                    for h in range(3):
                        for i in range(4):
                            ck = tb * 4 + i
                            tp2 = tpsum.tile([128, 128], F32, tag="tp",
                                             name=f"vt{h}_{ck}")
                            nc.tensor.transpose(
                                tp2[:, 0:64],
                                v_src[h][:, ck * 128:(ck + 1) * 128]
                                .bitcast(F32),
                                v_idn[h])
                            nc.vector.tensor_copy(
                                vs[h][:, ck * 65:ck * 65 + 64], tp2[:, 0:64])

            # ---------------- phase C/D: attention + out-proj -------------
            # stage list: (qb, h, g, gk, kb0, last)
            stages = []
            for qb in range(nqb):
                kbn = 2 * qb + 2
                ng = (kbn + 3) // 4
                for h in range(3):
                    for g in range(ng):
                        gk = min(4, kbn - g * 4)
                        stages.append((qb, h, g, gk, g * 4, g == ng - 1))
            nstages = len(stages)

            with (
                tc.tile_pool(name="spsum", bufs=2, space="PSUM") as spsum,
                tc.tile_pool(name="accpsum", bufs=1, space="PSUM") as accpsum,
                tc.tile_pool(name="bcpsum", bufs=1, space="PSUM") as bcpsum,
                tc.tile_pool(name="ypsum", bufs=2, space="PSUM") as ypsum,
                tc.tile_pool(name="ppool", bufs=3) as ppool,
                tc.tile_pool(name="apool", bufs=2) as apool,
                tc.tile_pool(name="rpool", bufs=2) as rpool,
                tc.tile_pool(name="ysb", bufs=3) as ysb,
            ):
                sp_t = {}    # stage idx -> S psum tile
                pt_t = {}    # stage idx -> P sbuf tile
                acc_t = {}   # (qb, h) -> acc psum tile [65, 256]
                sb_t = {}    # (qb, h) -> accsb sbuf tile
                attn = {}    # qb -> [3] attn tiles [64, 256]
                deferred = {}  # slot idx -> list of closures

                def defer(slot, fn):
                    deferred.setdefault(slot, []).append(fn)

                def emit_S(i):
                    qb, h, g, gk, kb0, last = stages[i]
                    sp = spsum.tile([128, gk * 256], F32, tag="s",
                                    name=f"s{qb}_{h}_{g}")
                    sp_t[i] = sp
                    q_sl = slice(qb * 256, (qb + 1) * 256)
                    for j in range(gk):
                        kbi = kb0 + j
                        nc.tensor.matmul(
                            sp[:, j * 256:(j + 1) * 256],
                            lhsT=k_ap(h)[:, kbi * 128:(kbi + 1) * 128],
                            rhs=q_ap(h)[:, q_sl],
                            start=True, stop=True)

                def emit_exp_mask(i):
                    qb, h, g, gk, kb0, last = stages[i]
                    pt = ppool.tile([128, gk * 256], F32R, tag="p",
                                    name=f"p{qb}_{h}_{g}")
                    pt_t[i] = pt
                    nc.scalar.activation(pt, sp_t[i], Act.Exp,
                                         bias=0.0, scale=0.125)
                    if last:
                        # diagonal kblocks are the last two of the unit:
                        # kbi = 2qb   -> keep iff u - si >= 0
                        # kbi = 2qb+1 -> keep iff u - si - 128 >= 0
                        for j in range(gk - 2, gk):
                            kbi = kb0 + j
                            base = qb * 256 - kbi * 128
                            v = pt[:, j * 256:(j + 1) * 256]
                            nc.gpsimd.affine_select(
                                out=v, in_=v,
                                compare_op=mybir.AluOpType.is_ge,
                                fill=0.0, base=base,
                                pattern=[[1, 256]], channel_multiplier=-1)

                def emit_PV(i):
                    qb, h, g, gk, kb0, last = stages[i]
                    if g == 0:
                        acc_t[(qb, h)] = accpsum.tile(
                            [65, 256], F32, tag="acc", name=f"acc{qb}_{h}")
                    acc = acc_t[(qb, h)]
                    kbn = 2 * qb + 2
                    pt = pt_t.pop(i)
                    for j in range(gk):
                        kbi = kb0 + j
                        nc.tensor.matmul(
                            acc,
                            lhsT=vs[h][:, kbi * 65:kbi * 65 + 65],
                            rhs=pt[:, j * 256:(j + 1) * 256],
                            start=(kbi == 0), stop=(kbi == kbn - 1))
                    sp_t.pop(i)

                def emit_epi1(qb, h):
                    # copy acc out of PSUM right away (frees the bank for
                    # the next head's accumulation)
                    accT = acc_t.pop((qb, h))
                    accsb = rpool.tile([65, 256], F32, tag="accsb",
                                       name=f"accsb{qb}_{h}")
                    nc.vector.tensor_copy(accsb, accT)
                    sb_t[(qb, h)] = accsb

                def emit_epi2(qb, h):
                    accsb = sb_t.pop((qb, h))
                    rec = rpool.tile([65, 256], F32R, tag="rec",
                                     name=f"rec{qb}_{h}")
                    with nc.allow_low_precision(
                            reason="fp32r operand rounding"):
                        nc.vector.reciprocal(rec[64:65], accsb[64:65])
                    bc = bcpsum.tile([64, 256], F32, tag="bc",
                                     name=f"bc{qb}_{h}")
                    nc.tensor.matmul(bc, lhsT=ones_t[64:65, :],
                                     rhs=rec[64:65, :],
                                     start=True, stop=True)
                    if h == 0:
                        attn[qb] = [apool.tile([64, 256], F32R,
                                               tag=f"attn{hh}",
                                               name=f"attn{hh}_{qb}")
                                    for hh in range(3)]
                    nc.vector.tensor_mul(attn[qb][h], accsb[0:64], bc)

                def emit_outproj(qb):
                    at = attn.pop(qb)
                    for mt in range(2):
                        t_sl = slice(mt * 128, (mt + 1) * 128)
                        row0 = qb * 256 + mt * 128
                        for nh in range(2):
                            n_sl = slice(nh * 384, (nh + 1) * 384)
                            yp = ypsum.tile([128, 384], F32, tag="y",
                                            name=f"y{qb}_{mt}_{nh}")
                            for h in range(3):
                                nc.tensor.matmul(yp, lhsT=at[h][:, t_sl],
                                                 rhs=wo_sb[h][:, n_sl],
                                                 start=(h == 0),
                                                 stop=(h == 2))
                            ys = ysb.tile([128, 384], F32, tag="ys",
                                          name=f"ys{qb}_{mt}_{nh}")
                            nc.vector.tensor_copy(ys, yp)
                            nc.sync.dma_start(
                                y_d[row0:row0 + 128, n_sl], ys)

                emit_S(0)
                for i in range(nstages):
                    qb, h, g, gk, kb0, last = stages[i]
                    if i + 1 < nstages:
                        emit_S(i + 1)
                    emit_exp_mask(i)
                    emit_PV(i)
                    if last:
                        emit_epi1(qb, h)
                    for fn in deferred.pop(i, ()):
                        fn()
                    if last:
                        defer(i + 1, lambda qb=qb, h=h: emit_epi2(qb, h))
                        if h == 2:
                            defer(i + 2, lambda qb=qb: emit_outproj(qb))
                for slot in sorted(deferred):
                    for fn in deferred[slot]:
                        fn()
    nc.compile()
    return nc


_PROG_CACHE = {}


def _get_program(t=T):
    if t not in _PROG_CACHE:
        _PROG_CACHE[t] = build_program(t)
    return _PROG_CACHE[t]


def make_in_maps(x, wq, bq, wk, bk, wv, bv, wo):
    in_maps = []
    for core in range(NCORES):
        b = core // 4
        hs = (core % 4) * HPC
        sl = [slice((hs + h) * DH, (hs + h + 1) * DH) for h in range(HPC)]
        # columns: q0,q1 | k0,k1 | q2,v0 | k2,v1 | v2
        cols = [wq[sl[0]].T, wq[sl[1]].T, wk[sl[0]].T, wk[sl[1]].T,
                wq[sl[2]].T, wv[sl[0]].T, wk[sl[2]].T, wv[sl[1]].T,
                wv[sl[2]].T]
        biases = [bq[sl[0]], bq[sl[1]], bk[sl[0]], bk[sl[1]],
                  bq[sl[2]], bv[sl[0]], bk[sl[2]], bv[sl[1]], bv[sl[2]]]
        wqkvT = np.ascontiguousarray(np.concatenate(cols, axis=1),
                                     dtype=np.float32)
        bqkv = np.ascontiguousarray(
            np.concatenate(biases)[:, None], dtype=np.float32)
        ch = slice(hs * DH, (hs + HPC) * DH)
        woT = np.ascontiguousarray(wo[:, ch].T, dtype=np.float32)
        in_maps.append({
            "x": np.ascontiguousarray(x[b], dtype=np.float32),
            "wqkvT": wqkvT,
            "bqkv": bqkv,
            "woT": woT,
        })
    return in_maps


def run(inputs, t=T, trace=False, **kw):
    """Run on hardware; returns (y, BassKernelResults)."""
    arrs = {k: np.asarray(v, dtype=np.float32) for k, v in inputs.items()}
    nc = _get_program(t)
    in_maps = make_in_maps(**arrs)
    res = run_bass_kernel_spmd(nc, in_maps, list(range(NCORES)),
                               trace=trace, **kw)
    outs = [np.asarray(m["y"], dtype=np.float32) for m in res.results]
    y = np.empty((B, t, EMBED), dtype=np.float32)
    for b in range(B):
        y[b] = outs[4 * b] + outs[4 * b + 1] + outs[4 * b + 2] + outs[4 * b + 3]
    return y, res


def kernel(**inputs):
    y, _ = run(inputs)
    return y
